# revision 7
# baseline (speedup 1.0000x reference)
"""MinEntropyConsensusLoss Trainium2 kernel (v2, bf16 fold pipeline).

loss = 0.5 * mean_b( min_c( -log_softmax(x)[b,c] - log_softmax(y)[b,c] ) )
     = 0.5 * mean_b( lse(x_b) + lse(y_b) - max_c(x[b,c] + y[b,c]) )

Uses max_c(x+y) = ln(max_c(exp(x)*exp(y))) so the exp tiles (needed for
lse anyway) feed the max path and no separate x+y add is ever computed.

Data-parallel over 8 NeuronCores; each core streams 16384 rows as 16
chunks of [128 partitions x 8 rows x 256 cols], DMA-bound at ~408 GB/s
(~5.1us/chunk). Per-chunk engine budget (measured rates):
  ACT    exp(x), exp(y) fp32->bf16 batched [128,2048]: 2 x ~2.0us
  GPSIMD p = ex * ey in bf16 [128,2048]: ~4.05us
  DVE    bf16 tensor_tensor runs at 2x (0.6ns/elem) but TENSOR_REDUCE
         is always 1x (1.04ns/elem) on TRN2 silicon, so reductions are
         binary FOLD chains (TT add/max over strided halves) down to 32
         elems/row, then one small reduce: sums ~2.7us, max ~1.35us.
The max chain for chunk k is emitted with the sum chain of chunk k+1
(one-chunk skew) so DVE never stalls on GPSIMD's product. The last
chunk runs as two half-chunks with the product on DVE to shorten the
pipeline drain. Stats accumulate as f32 [128, *]; tail does Ln + two
row-sum reduces + subtract -> [128,1] partial per core; host sums.

Hardware notes inherited from v1 (hold for future edits):
  - vector.tensor_tensor_reduce with op1=max/min WEDGES the core.
  - DMAs must issue from nc.sync's ring; ACT's HWDGE ring serializes
    DMAs behind its compute instructions.
  - ACT instructions cost ~190ns fixed overhead; READ_ACCUMULATOR
    costs ~280ns per accum_out (avoid).
  - bf16 in/out does NOT speed up DVE TENSOR_REDUCE (no 2x mode),
    only TENSOR_TENSOR with all-bf16 packed operands gets 2x.
"""

import sys

sys.path.insert(0, "/opt/trn_rl_repo")

import numpy as np

import concourse.bacc as bacc
import concourse.mybir as mybir
import concourse.tile as tile
from concourse.bass_utils import run_bass_kernel_spmd

B, C = 131072, 256
N_CORES = 8
R = B // N_CORES          # rows per core = 16384
T = 8                     # rows per partition per chunk
CH = T * C                # 2048 free elems per tensor per chunk
NCH = R // (128 * T)      # 16 chunks per core
NG = R // 128             # 128 row-groups (max values) per core
BUFS = 6                  # input tile ring depth
EBUFS = 3                 # combined-exp tile ring depth
PBUFS = 3                 # product tile ring depth

_cache = {}


def _build_nc(repeat=1):
    f32 = mybir.dt.float32
    bf16 = mybir.dt.bfloat16
    A = mybir.AluOpType
    Exp = mybir.ActivationFunctionType.Exp
    Ln = mybir.ActivationFunctionType.Ln
    X = mybir.AxisListType.X
    nc = bacc.Bacc("TRN2", target_bir_lowering=False, debug=False)
    x_d = nc.dram_tensor("x", [R, C], f32, kind="ExternalInput")
    y_d = nc.dram_tensor("y", [R, C], f32, kind="ExternalInput")
    out_d = nc.dram_tensor("out", [128, 1], f32, kind="ExternalOutput")

    # chunk c, partition p holds rows c*1024 + p*T + t (t contiguous)
    x_v = x_d.ap().rearrange("(c p t) f -> c p (t f)", c=NCH, p=128, t=T)
    y_v = y_d.ap().rearrange("(c p t) f -> c p (t f)", c=NCH, p=128, t=T)

    with tile.TileContext(nc) as tc:
        with (
            tc.tile_pool(name="xin", bufs=BUFS) as xin_pool,
            tc.tile_pool(name="yin", bufs=BUFS) as yin_pool,
            tc.tile_pool(name="exp", bufs=EBUFS) as e_pool,
            tc.tile_pool(name="prod", bufs=PBUFS) as p_pool,
            tc.tile_pool(name="fold", bufs=1) as f_pool,
            tc.tile_pool(name="stats", bufs=1) as stats_pool,
        ):
            sxy_t = stats_pool.tile([128, 2 * NG], f32, tag="sxy")
            mx_t = stats_pool.tile([128, NG], f32, tag="mx")

            def half(view_3d, f):
                # [128, t, 2f] -> two [128, t, f] halves
                t = view_3d.shape[1]
                v4 = view_3d.rearrange("p t (h f) -> p t h f", h=2, f=f)
                return v4[:, :, 0], v4[:, :, 1]

            def chain(src_view, nt, op, red, out_slice, tpre):
                # src_view: [128, nt*256] 2D AP; three bf16 TT folds to
                # 32 elems/row then one reduce into f32 out_slice.
                # Fold tiles are fixed max-size (nt<=16 for sums, <=8 for
                # maxes) in a bufs=1 pool: safe, all DVE in-order.
                v = src_view.rearrange("p (t f) -> p t f", t=nt)
                t1 = f_pool.tile([128, 16 * 128], bf16, tag=f"{tpre}1")
                t2 = f_pool.tile([128, 16 * 64], bf16, tag=f"{tpre}2")
                t3 = f_pool.tile([128, 16 * 32], bf16, tag=f"{tpre}3")
                a, b = half(v, 128)
                o1 = t1[:, : nt * 128].rearrange("p (t f) -> p t f", t=nt)
                nc.vector.tensor_tensor(out=o1, in0=a, in1=b, op=op)
                a, b = half(o1, 64)
                o2 = t2[:, : nt * 64].rearrange("p (t f) -> p t f", t=nt)
                nc.vector.tensor_tensor(out=o2, in0=a, in1=b, op=op)
                a, b = half(o2, 32)
                o3 = t3[:, : nt * 32].rearrange("p (t f) -> p t f", t=nt)
                nc.vector.tensor_tensor(out=o3, in0=a, in1=b, op=op)
                red(out_slice, o3, axis=X)

            def sum_chain(e_view, nt, c_out):
                chain(e_view, nt, A.add, nc.vector.reduce_sum,
                      sxy_t[:, c_out : c_out + nt], "s")

            def max_chain(p_view, nt, c_out):
                chain(p_view, nt, A.max, nc.vector.reduce_max,
                      mx_t[:, c_out : c_out + nt], "m")

            def one_pass():
                pending_max = None  # (p_t, nt, c_out) skewed by one chunk
                for c in range(NCH - 1):
                    h = CH // 2
                    x_t = xin_pool.tile([128, CH], f32, tag="x")
                    y_t = yin_pool.tile([128, CH], f32, tag="y")
                    nc.sync.dma_start(x_t[:, :h], x_v[c][:, :h])
                    nc.sync.dma_start(x_t[:, h:], x_v[c][:, h:])
                    nc.sync.dma_start(y_t[:, :h], y_v[c][:, :h])
                    nc.sync.dma_start(y_t[:, h:], y_v[c][:, h:])

                    e_t = e_pool.tile([128, 2 * CH], bf16, tag="e")
                    nc.scalar.activation(e_t[:, :CH], x_t[:], Exp)
                    nc.scalar.activation(e_t[:, CH:], y_t[:], Exp)

                    p_t = p_pool.tile([128, CH], bf16, tag="p")
                    nc.gpsimd.tensor_tensor(
                        out=p_t[:], in0=e_t[:, :CH], in1=e_t[:, CH:], op=A.mult
                    )

                    sum_chain(e_t[:], 2 * T, 2 * T * c)
                    if pending_max is not None:
                        max_chain(*pending_max)
                    pending_max = (p_t[:], T, T * c)

                # last chunk as two half-chunks (4 rows each), product on
                # DVE, no skew: shortens the pipeline drain. Reuses the
                # full-size tile tags (partial use) to save SBUF.
                c = NCH - 1
                hq = CH // 2
                for hh in range(2):
                    xs = x_v[c][:, hh * hq : (hh + 1) * hq]
                    ys = y_v[c][:, hh * hq : (hh + 1) * hq]
                    x_t = xin_pool.tile([128, CH], f32, tag="x")
                    y_t = yin_pool.tile([128, CH], f32, tag="y")
                    nc.sync.dma_start(x_t[:, :hq], xs)
                    nc.sync.dma_start(y_t[:, :hq], ys)

                    e_t = e_pool.tile([128, 2 * CH], bf16, tag="e")
                    nc.scalar.activation(e_t[:, :hq], x_t[:, :hq], Exp)
                    nc.scalar.activation(e_t[:, hq : 2 * hq], y_t[:, :hq], Exp)

                    if pending_max is not None:
                        max_chain(*pending_max)
                        pending_max = None

                    p_t = p_pool.tile([128, CH], bf16, tag="p")
                    nc.vector.tensor_tensor(
                        out=p_t[:, :hq], in0=e_t[:, :hq],
                        in1=e_t[:, hq : 2 * hq], op=A.mult,
                    )
                    sum_chain(e_t[:, : 2 * hq], T, 2 * T * c + T * hh)
                    max_chain(p_t[:, :hq], T // 2, T * c + (T // 2) * hh)

            if repeat > 1:
                with tc.For_i(0, repeat, 1):
                    one_pass()
            else:
                one_pass()

            # --- device tail: stats -> [128, 1] partial sum ---
            ln_t = stats_pool.tile([128, 2 * NG], f32, tag="ln")
            mln_t = stats_pool.tile([128, NG], f32, tag="mln")
            ts_t = stats_pool.tile([128, 1], f32, tag="ts")
            tm_t = stats_pool.tile([128, 1], f32, tag="tm")
            o_t = stats_pool.tile([128, 1], f32, tag="o")
            nc.scalar.activation(ln_t[:], sxy_t[:], Ln)
            # mx holds max(ex*ey) = exp(max(x+y)); ln() recovers the max
            nc.scalar.activation(mln_t[:], mx_t[:], Ln)
            nc.vector.reduce_sum(ts_t[:], ln_t[:], axis=X)
            nc.vector.reduce_sum(tm_t[:], mln_t[:], axis=X)
            nc.vector.tensor_tensor(out=o_t[:], in0=ts_t[:], in1=tm_t[:], op=A.subtract)
            nc.sync.dma_start(out_d.ap(), o_t[:])

    nc.compile()
    return nc


def get_nc():
    if "nc" not in _cache:
        _cache["nc"] = _build_nc()
    return _cache["nc"]


def run_cores(x, y, **kw):
    nc = get_nc()
    x = np.ascontiguousarray(np.asarray(x, dtype=np.float32))
    y = np.ascontiguousarray(np.asarray(y, dtype=np.float32))
    in_maps = [
        {"x": x[k * R : (k + 1) * R], "y": y[k * R : (k + 1) * R]}
        for k in range(N_CORES)
    ]
    return run_bass_kernel_spmd(nc, in_maps, list(range(N_CORES)), **kw)


def kernel(x, y):
    res = run_cores(x, y)
    total = 0.0
    for r in res.results:
        total += float(np.sum(r["out"].astype(np.float64)))
    return np.float32(0.5 * total / B)


# revision 8
# speedup vs baseline: 1.2448x; 1.2448x over previous
"""MinEntropyConsensusLoss Trainium2 kernel (v2, bf16 fold pipeline).

loss = 0.5 * mean_b( min_c( -log_softmax(x)[b,c] - log_softmax(y)[b,c] ) )
     = 0.5 * mean_b( lse(x_b) + lse(y_b) - max_c(x[b,c] + y[b,c]) )

Uses max_c(x+y) = ln(max_c(exp(x)*exp(y))) so the exp tiles (needed for
lse anyway) feed the max path and no separate x+y add is ever computed.

Data-parallel over 8 NeuronCores; each core streams 16384 rows as 16
chunks of [128 partitions x 8 rows x 256 cols], DMA-bound at ~408 GB/s
(~5.1us/chunk). Per-chunk engine budget (measured rates):
  ACT    exp(x), exp(y) fp32->bf16 batched [128,2048]: 2 x ~2.0us
  GPSIMD p = ex * ey in bf16 [128,2048]: ~4.05us
  DVE    bf16 tensor_tensor runs at 2x (0.6ns/elem) but TENSOR_REDUCE
         is always 1x (1.04ns/elem) on TRN2 silicon, so reductions are
         binary FOLD chains (TT add/max over strided halves) down to 32
         elems/row, then one small reduce: sums ~2.7us, max ~1.35us.
The max chain for chunk k is emitted with the sum chain of chunk k+1
(one-chunk skew) so DVE never stalls on GPSIMD's product. The last
chunk runs as two half-chunks with the product on DVE to shorten the
pipeline drain. Stats accumulate as f32 [128, *]; tail does Ln + two
row-sum reduces + subtract -> [128,1] partial per core; host sums.

Hardware notes inherited from v1 (hold for future edits):
  - vector.tensor_tensor_reduce with op1=max/min WEDGES the core.
  - DMAs must issue from nc.sync's ring; ACT's HWDGE ring serializes
    DMAs behind its compute instructions.
  - ACT instructions cost ~190ns fixed overhead; READ_ACCUMULATOR
    costs ~280ns per accum_out (avoid).
  - bf16 in/out does NOT speed up DVE TENSOR_REDUCE (no 2x mode),
    only TENSOR_TENSOR with all-bf16 packed operands gets 2x.
"""

import sys

sys.path.insert(0, "/opt/trn_rl_repo")

import numpy as np

import concourse.bacc as bacc
import concourse.mybir as mybir
import concourse.tile as tile
from concourse.bass_utils import run_bass_kernel_spmd

B, C = 131072, 256
N_CORES = 8
R = B // N_CORES          # rows per core = 16384
T = 8                     # rows per partition per chunk
CH = T * C                # 2048 free elems per tensor per chunk
NCH = R // (128 * T)      # 16 chunks per core
NG = R // 128             # 128 row-groups (max values) per core
BUFS = 6                  # input tile ring depth
EBUFS = 3                 # combined-exp tile ring depth
PBUFS = 3                 # product tile ring depth

_cache = {}


def _build_nc(repeat=1):
    f32 = mybir.dt.float32
    bf16 = mybir.dt.bfloat16
    A = mybir.AluOpType
    Exp = mybir.ActivationFunctionType.Exp
    Ln = mybir.ActivationFunctionType.Ln
    X = mybir.AxisListType.X
    nc = bacc.Bacc("TRN2", target_bir_lowering=False, debug=False)
    x_d = nc.dram_tensor("x", [R, C], f32, kind="ExternalInput")
    y_d = nc.dram_tensor("y", [R, C], f32, kind="ExternalInput")
    out_d = nc.dram_tensor("out", [128, 1], f32, kind="ExternalOutput")

    # chunk c, partition p holds rows c*1024 + p*T + t (t contiguous)
    x_v = x_d.ap().rearrange("(c p t) f -> c p (t f)", c=NCH, p=128, t=T)
    y_v = y_d.ap().rearrange("(c p t) f -> c p (t f)", c=NCH, p=128, t=T)

    with tile.TileContext(nc) as tc:
        with (
            tc.tile_pool(name="xin", bufs=BUFS) as xin_pool,
            tc.tile_pool(name="yin", bufs=BUFS) as yin_pool,
            tc.tile_pool(name="exp", bufs=EBUFS) as e_pool,
            tc.tile_pool(name="prod", bufs=PBUFS) as p_pool,
            tc.tile_pool(name="fold", bufs=1) as f_pool,
            tc.tile_pool(name="stats", bufs=1) as stats_pool,
        ):
            sxy_t = stats_pool.tile([128, 2 * NG], f32, tag="sxy")
            mx_t = stats_pool.tile([128, NG], f32, tag="mx")

            def half(view_3d, f):
                # [128, t, 2f] -> two [128, t, f] halves
                t = view_3d.shape[1]
                v4 = view_3d.rearrange("p t (h f) -> p t h f", h=2, f=f)
                return v4[:, :, 0], v4[:, :, 1]

            def chain(src_view, nt, op, red, out_slice, tpre):
                # src_view: [128, nt*256] 2D AP; three bf16 TT folds to
                # 32 elems/row then one reduce into f32 out_slice.
                # Fold tiles are fixed max-size (nt<=16 for sums, <=8 for
                # maxes) in a bufs=1 pool: safe, all DVE in-order.
                v = src_view.rearrange("p (t f) -> p t f", t=nt)
                t1 = f_pool.tile([128, 16 * 128], bf16, tag=f"{tpre}1")
                t2 = f_pool.tile([128, 16 * 64], bf16, tag=f"{tpre}2")
                t3 = f_pool.tile([128, 16 * 32], bf16, tag=f"{tpre}3")
                a, b = half(v, 128)
                o1 = t1[:, : nt * 128].rearrange("p (t f) -> p t f", t=nt)
                nc.vector.tensor_tensor(out=o1, in0=a, in1=b, op=op)
                a, b = half(o1, 64)
                o2 = t2[:, : nt * 64].rearrange("p (t f) -> p t f", t=nt)
                nc.vector.tensor_tensor(out=o2, in0=a, in1=b, op=op)
                a, b = half(o2, 32)
                o3 = t3[:, : nt * 32].rearrange("p (t f) -> p t f", t=nt)
                nc.vector.tensor_tensor(out=o3, in0=a, in1=b, op=op)
                red(out_slice, o3, axis=X)

            def sum_chain(e_view, nt, c_out):
                chain(e_view, nt, A.add, nc.vector.reduce_sum,
                      sxy_t[:, c_out : c_out + nt], "s")

            def max_chain(p_view, nt, c_out):
                chain(p_view, nt, A.max, nc.vector.reduce_max,
                      mx_t[:, c_out : c_out + nt], "m")

            def one_pass():
                for c in range(NCH - 1):
                    h = CH // 2
                    x_t = xin_pool.tile([128, CH], f32, tag="x")
                    y_t = yin_pool.tile([128, CH], f32, tag="y")
                    nc.sync.dma_start(x_t[:, :h], x_v[c][:, :h])
                    nc.sync.dma_start(x_t[:, h:], x_v[c][:, h:])
                    nc.sync.dma_start(y_t[:, :h], y_v[c][:, :h])
                    nc.sync.dma_start(y_t[:, h:], y_v[c][:, h:])

                    e_t = e_pool.tile([128, 2 * CH], bf16, tag="e")
                    nc.scalar.activation(e_t[:, :CH], x_t[:], Exp)
                    nc.scalar.activation(e_t[:, CH:], y_t[:], Exp)

                    # product on DVE: GPSIMD's software TENSOR_TENSOR
                    # stalls concurrent DVE instructions to ~zero
                    # throughput (measured), so GPSIMD is unusable here.
                    p_t = p_pool.tile([128, CH], bf16, tag="p")
                    nc.vector.tensor_tensor(
                        out=p_t[:], in0=e_t[:, :CH], in1=e_t[:, CH:], op=A.mult
                    )

                    sum_chain(e_t[:], 2 * T, 2 * T * c)
                    max_chain(p_t[:], T, T * c)

                # last chunk as two half-chunks (4 rows each) to shorten
                # the pipeline drain. Reuses the full-size tile tags
                # (partial use) to save SBUF.
                c = NCH - 1
                hq = CH // 2
                for hh in range(2):
                    xs = x_v[c][:, hh * hq : (hh + 1) * hq]
                    ys = y_v[c][:, hh * hq : (hh + 1) * hq]
                    x_t = xin_pool.tile([128, CH], f32, tag="x")
                    y_t = yin_pool.tile([128, CH], f32, tag="y")
                    nc.sync.dma_start(x_t[:, :hq], xs)
                    nc.sync.dma_start(y_t[:, :hq], ys)

                    e_t = e_pool.tile([128, 2 * CH], bf16, tag="e")
                    nc.scalar.activation(e_t[:, :hq], x_t[:, :hq], Exp)
                    nc.scalar.activation(e_t[:, hq : 2 * hq], y_t[:, :hq], Exp)

                    p_t = p_pool.tile([128, CH], bf16, tag="p")
                    nc.vector.tensor_tensor(
                        out=p_t[:, :hq], in0=e_t[:, :hq],
                        in1=e_t[:, hq : 2 * hq], op=A.mult,
                    )
                    sum_chain(e_t[:, : 2 * hq], T, 2 * T * c + T * hh)
                    max_chain(p_t[:, :hq], T // 2, T * c + (T // 2) * hh)

            if repeat > 1:
                with tc.For_i(0, repeat, 1):
                    one_pass()
            else:
                one_pass()

            # --- device tail: stats -> [128, 1] partial sum ---
            ln_t = stats_pool.tile([128, 2 * NG], f32, tag="ln")
            mln_t = stats_pool.tile([128, NG], f32, tag="mln")
            ts_t = stats_pool.tile([128, 1], f32, tag="ts")
            tm_t = stats_pool.tile([128, 1], f32, tag="tm")
            o_t = stats_pool.tile([128, 1], f32, tag="o")
            nc.scalar.activation(ln_t[:], sxy_t[:], Ln)
            # mx holds max(ex*ey) = exp(max(x+y)); ln() recovers the max
            nc.scalar.activation(mln_t[:], mx_t[:], Ln)
            nc.vector.reduce_sum(ts_t[:], ln_t[:], axis=X)
            nc.vector.reduce_sum(tm_t[:], mln_t[:], axis=X)
            nc.vector.tensor_tensor(out=o_t[:], in0=ts_t[:], in1=tm_t[:], op=A.subtract)
            nc.sync.dma_start(out_d.ap(), o_t[:])

    nc.compile()
    return nc


def get_nc():
    if "nc" not in _cache:
        _cache["nc"] = _build_nc()
    return _cache["nc"]


def run_cores(x, y, **kw):
    nc = get_nc()
    x = np.ascontiguousarray(np.asarray(x, dtype=np.float32))
    y = np.ascontiguousarray(np.asarray(y, dtype=np.float32))
    in_maps = [
        {"x": x[k * R : (k + 1) * R], "y": y[k * R : (k + 1) * R]}
        for k in range(N_CORES)
    ]
    return run_bass_kernel_spmd(nc, in_maps, list(range(N_CORES)), **kw)


def kernel(x, y):
    res = run_cores(x, y)
    total = 0.0
    for r in res.results:
        total += float(np.sum(r["out"].astype(np.float64)))
    return np.float32(0.5 * total / B)
